# revision 25
# baseline (speedup 1.0000x reference)
"""Multi-head attention Trainium2 Bass kernel (v2 — woven pipeline).

Problem: q,k,v [B=4, H=16, N=2048, D=64] fp32 ->
         out [B, N, H*D] = softmax(q @ k^T / sqrt(D)) @ v, heads concatenated.

Sharding: B*H = 64 (b,h) pairs split across 8 cores -> 8 heads/core (data/head
parallel, no collectives). Each core runs the same SPMD program on its own
q/k/v shard [8, 2048, 64]; the host reassembles [4, 2048, 1024].

Design (per core). ScalarE (ACT) is the hard floor: 8 heads x 32 exp
instructions of [128, 1024] at ~(1024+352)/1.2 ns = ~294us busy. Everything
else is arranged so ACT never waits:

  - prep(h): q,k cast fp32->bf16 into a DRAM scratch [2048,128]=[q|k] (SP
    queue), one xbar DMA-transpose -> qkT [128,2048] sbuf (Q^T on partitions
    0..63, K^T on 64..127), partition-swapped copy qkT_sw; v loads via SWDGE.
  - S^T phase, head h: 32 steps (ic,j); each step: 2 matmuls (K=64, N=512,
    alternating PE row groups via tile_position) -> st PSUM [128,1024], then
    one ACT exp (scale fused) -> exps sbuf bf16. st pool is double-buffered
    so PE runs up to 2 steps ahead of ACT.
  - AV phase of head h-1 is WOVEN into head h's S^T steps (the PE queue
    carries [av mms][st mms] per step) so ACT keeps consuming st tiles while
    the PE retires the previous head's AV work.  AV uses out^T layout:
    av [65, 512] += [V|1]_j^T @ expS^T_j with chunk-PAIR accumulators so one
    LDWEIGHTS of V_j serves 2 matmuls.  PSUM row 64 = softmax denominator.
  - Finish (no PE transpose): DVE reciprocal of the denominator row,
    GPSIMD partition_broadcast of the reciprocal to 64 partitions, DVE
    multiply -> out^T [64, 2048] fp32, DMA'd to DRAM as [HPC, D, N].
    The host does the final [D,N] -> [N,D] transpose during reassembly
    (it is doing a head-concat transpose there anyway).

Engine budget per core: ACT ~294us (bottleneck), PE ~250us (S^T 64 + AV 64
matmuls + ldweights per head), DVE ~10us, Pool ~15us, DMA ~85us spread over
SP + 8 SWDGE rings.
"""

import os
import sys

sys.path.insert(0, "/opt/trn_rl_repo")

import numpy as np

try:  # persistent XLA executable cache: skips NEFF recompiles across processes
    import jax

    jax.config.update("jax_compilation_cache_dir", "/root/.cache/jax_bass")
    jax.config.update("jax_persistent_cache_min_compile_time_secs", 1.0)
    jax.config.update("jax_persistent_cache_min_entry_size_bytes", 0)
except Exception:
    pass

import concourse.bass as bass
import concourse.mybir as mybir
import concourse.tile as tile
from concourse import bacc
from concourse import library_config
from concourse.bass_utils import run_bass_kernel_spmd

B, H, N, D = 4, 16, 2048, 64
NCORES = 8
HPC = (B * H) // NCORES  # heads per core
NT = N // 128  # 16 j-tiles per head
SCALE = float(D) ** -0.5
F32 = mybir.dt.float32
BF16 = mybir.dt.bfloat16


def build_nc(reps: int = 1, variant: str = "full"):
    nc = bacc.Bacc("TRN2", target_bir_lowering=False, debug=False, num_devices=NCORES)
    q = nc.dram_tensor("q", [HPC, N, D], F32, kind="ExternalInput").ap()
    k = nc.dram_tensor("k", [HPC, N, D], F32, kind="ExternalInput").ap()
    v = nc.dram_tensor("v", [HPC, N, D], F32, kind="ExternalInput").ap()
    # out^T layout: [head, d, n] — host transposes to [head, n, d]
    out = nc.dram_tensor("out", [HPC, D, N], F32, kind="ExternalOutput").ap()

    with tile.TileContext(nc) as tc:
        nc.gpsimd.load_library(library_config.attn)
        with (
            tc.tile_pool(name="const", bufs=1) as const_pool,
            tc.tile_pool(name="qtkt", bufs=2) as qtkt,
            tc.tile_pool(name="exps", bufs=2) as exps_pool,
            tc.tile_pool(name="vb", bufs=2) as vb_pool,
            tc.tile_pool(name="stage", bufs=2) as stage_pool,
            tc.tile_pool(name="stgb", bufs=2) as stgb_pool,
            tc.tile_pool(name="outT", bufs=2) as outT_pool,
            tc.tile_pool(name="fin", bufs=2) as fin_pool,
            tc.tile_pool(name="st", bufs=3, space="PSUM") as st_pool,
            tc.tile_pool(name="av", bufs=2, space="PSUM") as av_pool,
            tc.tile_pool(name="dram", bufs=2, space="DRAM") as dram_pool,
        ):
            dummy_exps = None
            if variant == "dumbav":
                dummy_exps = const_pool.tile([128, 1, N], BF16)
                nc.vector.memset(dummy_exps[:], 0.5)

            def prep(h):
                """Load q/k/v for head h with NO gpsimd (Pool/Q7) involvement
                — the Pool queue is reserved for the partition_broadcast
                finish work.  q,k,v load fp32 via the SP hardware DGE and are
                cast to bf16 on DVE.  v casts straight into vb (SBUF only);
                q,k go through a bf16 DRAM scratch so the xbar DMA-transpose
                can produce qt/kt [128, 2048].  Rows 64..127 of qt/kt and the
                ones column of vb are initialized ONCE outside the reps loop
                (pool slots are stable), so S^T matmuls contract over K=128
                zero padding — a homogeneous PE instruction stream."""
                sv = stage_pool.tile([128, NT, D], F32, tag="sv")
                nc.sync.dma_start(sv[:], v[h].rearrange("(t p) d -> p t d", p=128))
                vb = vb_pool.tile([128, NT, D + 1], BF16, tag="vb")
                nc.vector.tensor_copy(vb[:, :, 0:D], sv[:])
                nc.vector.memset(vb[:, :, D : D + 1], 1.0)

                qt = qtkt.tile([128, N], BF16, tag="qt")
                kt = qtkt.tile([128, N], BF16, tag="kt")
                nc.vector.memset(qt[64:128, :], 0.0)
                nc.vector.memset(kt[64:128, :], 0.0)
                for src_t, dst, dtag in ((q, qt, "scrq"), (k, kt, "scrk")):
                    stg = stage_pool.tile([128, NT, D], F32, tag="sqk")
                    nc.sync.dma_start(
                        stg[:], src_t[h].rearrange("(t p) d -> p t d", p=128)
                    )
                    stb = stgb_pool.tile([128, NT, D], BF16, tag="sqkb")
                    nc.vector.tensor_copy(stb[:], stg[:])
                    scr = dram_pool.tile([N, D], BF16, tag=dtag)
                    nc.sync.dma_start(
                        scr.rearrange("(t p) d -> p t d", p=128), stb[:]
                    )
                    nc.sync.dma_start_transpose(dst[0:64, :], scr[:])
                return qt, kt, vb

            def emit_av_step(weave, s, st_ctx):
                """One step of the previous head's AV work.  Chunks run
                sequentially (chunk c = s//8 accumulates its 16 j-matmuls two
                per step); at a chunk's last step the reciprocal-broadcast-
                multiply finish runs, and the head's out^T DMA at s==31."""
                exps_p, vb_p, hp = weave
                c, sj = divmod(s, NT // 2)
                if sj == 0:
                    av_t = av_pool.tile([D + 1, 512], F32, tag="av")
                    st_ctx["av"] = av_t
                    if c == 0:
                        outT_t = outT_pool.tile([D, N], F32, tag="outT")
                        st_ctx["outT"] = outT_t
                av = st_ctx["av"]
                for jj in (2 * sj, 2 * sj + 1):
                    if variant == "dumbav":
                        src_ap = dummy_exps[:, 0, c * 512 : (c + 1) * 512]
                    else:
                        src_ap = exps_p[:, jj, c * 512 : (c + 1) * 512]
                    nc.tensor.matmul(
                        av[:],
                        vb_p[:, jj, :],
                        src_ap,
                        start=(jj == 0),
                        stop=(jj == NT - 1),
                    )
                if sj == NT // 2 - 1:
                    outT = st_ctx["outT"]
                    rcp = fin_pool.tile([1, 512], F32, tag="rcp")
                    nc.vector.reciprocal(rcp[:], av[D : D + 1, :])
                    rb = fin_pool.tile([D, 512], F32, tag="rb")
                    nc.gpsimd.partition_broadcast(rb[:], rcp[:])
                    nc.vector.tensor_mul(
                        outT[:, c * 512 : (c + 1) * 512], av[0:D, :], rb[:]
                    )
                    if c == 3:
                        nc.sync.dma_start(out[hp], outT[:])

            # prep(0) is seeded OUTSIDE the reps loop; each body preps the
            # NEXT rep's head 0 at its last head (8 preps/body keeps the
            # pool-slot rotation aligned so rep r+1's head 0 reads the slots
            # the body-end prep wrote).  This overlaps the rep-boundary
            # prep chain under the previous rep's AV tail.
            prep_q = [prep(0)]

            def body():
                skip_st = variant == "preponly"
                skip_av = variant in ("stonly", "preponly")
                weave = None
                BURST = int(os.environ.get("BURST", "1"))
                for h in range(HPC):
                    qt, kt, vb = prep_q.pop(0)
                    exps = exps_pool.tile([128, NT, N], BF16, tag="exps")
                    st_ctx = {}
                    for g in range(0, 2 * NT, BURST):
                        for s in range(g, g + BURST):
                            if s == 8:
                                prep_q.append(prep((h + 1) % HPC))
                            if skip_st:
                                continue
                            ic, j = divmod(s, NT)
                            st = st_pool.tile([128, 1024], F32, tag="st")
                            for m in range(2):
                                i0 = ic * 1024 + m * 512
                                nc.tensor.matmul(
                                    st[:, m * 512 : (m + 1) * 512],
                                    kt[:, j * 128 : (j + 1) * 128],
                                    qt[:, i0 : i0 + 512],
                                    start=True,
                                    stop=True,
                                )
                            nc.scalar.activation(
                                exps[:, j, ic * 1024 : (ic + 1) * 1024],
                                st[:],
                                mybir.ActivationFunctionType.Exp,
                                scale=SCALE,
                            )
                        if weave is not None and not skip_av:
                            for s in range(g, g + BURST):
                                emit_av_step(weave, s, st_ctx)
                    weave = (exps, vb, h)
                # tail: last head's AV with no st stream to weave under
                if not skip_av:
                    st_ctx = {}
                    for s in range(2 * NT):
                        emit_av_step(weave, s, st_ctx)

            if reps == 1:
                body()
            else:
                tc.For_i_unrolled(0, reps, 1, lambda iv: body(), max_unroll=1)

    nc.compile()
    return nc


_NC_CACHE: dict = {}


def get_nc(reps: int = 1, variant: str = "full"):
    key = (reps, variant)
    if key not in _NC_CACHE:
        _NC_CACHE[key] = build_nc(reps, variant)
    return _NC_CACHE[key]


def shard_inputs(q: np.ndarray, k: np.ndarray, v: np.ndarray):
    qr = np.ascontiguousarray(q.reshape(B * H, N, D))
    kr = np.ascontiguousarray(k.reshape(B * H, N, D))
    vr = np.ascontiguousarray(v.reshape(B * H, N, D))
    in_maps = []
    for c in range(NCORES):
        s = slice(c * HPC, (c + 1) * HPC)
        in_maps.append(
            {
                "q": np.ascontiguousarray(qr[s]),
                "k": np.ascontiguousarray(kr[s]),
                "v": np.ascontiguousarray(vr[s]),
            }
        )
    return in_maps


def assemble_output(results) -> np.ndarray:
    shards = np.stack([results[c]["out"] for c in range(NCORES)])  # [8, HPC, D, N]
    full = shards.reshape(B, H, D, N)
    return np.ascontiguousarray(full.transpose(0, 3, 1, 2).reshape(B, N, H * D))


def kernel(q: np.ndarray, k: np.ndarray, v: np.ndarray) -> np.ndarray:
    nc = get_nc(reps=1)
    in_maps = shard_inputs(q, k, v)
    res = run_bass_kernel_spmd(nc, in_maps, core_ids=list(range(NCORES)))
    return assemble_output(res.results)


if __name__ == "__main__":
    rng = np.random.default_rng(0)
    q = rng.standard_normal((B, H, N, D), dtype=np.float32)
    k = rng.standard_normal((B, H, N, D), dtype=np.float32)
    v = rng.standard_normal((B, H, N, D), dtype=np.float32)
    o = kernel(q, k, v)
    print(o.shape, o.dtype)
